# revision 39
# baseline (speedup 1.0000x reference)
"""Trainium2 kernel for nn_MeanSquaredError2: MSE between argmax-decoded
heatmap coordinates and targets.

loss = sum_{b,j} [(px - tpx)^2 + (py - tpy)^2] / (B*NJ)
  where idx = argmax(h[b,j]), px = (idx%14)/16, py = (idx//14)/16 and
  (tpx, tpy) follow the reference's concat-then-reshape pairing of t.
Inputs o and v do not affect the result (USE_VISIBILITY=False).

Pure data parallel over 8 cores (2048 batches each). Per core, h streams in
16 tiles of [128 part x (14 rows x 196 pix)]; the whole
quantize+pack+row-argmax runs as ONE single-stream custom DVE instruction
per tile (~1.07 cyc/elem), leaving the kernel DMA-bound:

  custom op QPACK_ROWMAX_ANT (registered into dve_ops.OPS at build time):
      u  = (h*2^16 + MAGIC)        # rounds h*2^16 to the 256-grid
      q  = u - MAGIC               # exact (Sterbenz): q = 256*n
      k  = (q - Idx) + PageIdx(0, 196+2^20)
                                   # == q - j + s*2^20 for in-page pixel j,
                                   #    row (joint) s; all exact (< 2^24)
      out = running_max(k)         # inclusive scan, no page reset needed:
                                   # s*2^20 > range(q-j) isolates rows

  The position rides in k's low byte (-j mod 256) and the row offset above
  the value bits, so the running max at element s*196+195 equals row s's
  packed max; that element is extracted per row (ACT copy, strided). The
  "-j + s*PSTEP" counter is one page-counter scan whose steady state is
  hand-patched from BYPASS(CURR) to SUBTRACT(CURR, One) (the documented
  UopConfig escape hatch), and the outer max-scan's expr contains that scan
  - Scan.__post_init__ forbids the composition, but lower() schedules it
  correctly; both validated against numpy on hardware.

Tail (4 chunks of [128,56], interleaved with the stream; 3 more custom DVE
ops): MODJ_ANT: wc = E - 256*round(E/256), j = 256*(wc>0) - wc;
DECX_ANT: cand = round(j/14), xr = j - 14*cand, x = xr + 14*(xr<0);
DECY_ANT: y = cand - (xr<0); then dx = x/16 - tx, dy = y/16 - ty (DVE stt)
and ACT Square+accum per partition. The last h tile streams as two k-halves
to halve the final serial QPACK. Host sums 8x[128,8] partials / N.

Quantization is 2^-8 (vs jnp.argmax exact): ~1% of rows hit top-2 ties and
may decode the runner-up pixel; the loss deltas are zero-mean and mostly
cancel, measured rel err ~1e-3 (threshold 2e-2).
"""
import numpy as np

B = 16384
NJ = 14
NPIX = 196
N_CORES = 8
ROWS_PER_TILE = 1792          # 128 partitions x 14 rows
K_PER_PART = 14
N_TILES = 16                  # (B/N_CORES)*NJ / ROWS_PER_TILE

SCALE = float(2 ** 16)
MAGIC = 1.5 * 2 ** 32         # ulp = 256 -> q on the 256-grid
M3 = 1.5 * 2 ** 23            # ulp = 1   -> round-to-integer magic
PSTEP = float(2 ** 20)        # per-row offset; > range(q - j) ~ 2*6.3*2^16

_STATE = {}


def _register_op(name, spec, subdim, patch=None):
    """Register one custom DVE op into dve_ops.OPS at runtime; `patch` may
    hand-edit the lowered uop list (the documented escape hatch for programs
    `lower()` cannot express directly)."""
    import concourse.dve_ops as dve_ops
    from concourse.dve_ops import DveOp
    from concourse.dve_spec import _has_src1 as has_src1
    from concourse.dve_uop import DveOpSpec

    if name in dve_ops._SUB_OPCODE_FOR_NAME:
        return next(op for op in dve_ops.OPS if op.name == name)
    row = dve_ops._CUSTOM_DVE_ROW_BASE + len(dve_ops.OPS)
    assert row < 0x20, "custom-DVE opcode rows exhausted"
    from concourse.dve_spec import lower
    shas = {}
    compiled = {}
    for ver in ("v3", "v4"):
        uops = lower(spec, ver=ver)
        if patch is not None:
            patch(uops)
        s = DveOpSpec(name=name, opcode=row, uops=uops, rd1_en=has_src1(spec))
        shas[ver] = s.sha(ver)
        compiled[ver] = s
    op = DveOp(name, spec, subdim=subdim, uops_sha=shas)
    dve_ops.OPS.append(op)
    dve_ops._SUB_OPCODE_FOR_NAME[name] = row
    dve_ops.CUSTOM_DVE_SPECS[name] = spec
    for ver, s in compiled.items():
        dve_ops._COMPILE_CACHE[(name, ver)] = s
    return op


def _register_qpack():
    from concourse.dve_spec import (
        Spec, Src0, C0, C1, C2, Zero, One, Scan,
    )
    from concourse.dve_uop import AluOp, AluInp

    def _ref(in0, in1, c0, c1, c2):
        p = in0.shape[0]
        flat = in0.reshape(p, -1)
        n = flat.shape[1]
        u = (flat.astype(np.float32) * np.float32(c0)).astype(np.float32)
        u = (u + np.float32(c1)).astype(np.float32)
        q = (u - np.float32(c1)).astype(np.float32)
        j = (np.arange(n) % NPIX).astype(np.float32)
        s = (np.arange(n) // NPIX).astype(np.float32) * np.float32(c2 - NPIX + 1)
        k = (q + (s - j).astype(np.float32)).astype(np.float32)
        return np.maximum.accumulate(k, axis=1).reshape(in0.shape)

    u = (Src0 * C0) + C1
    q = u - C1
    # page-counter scan (PageIdx(One, C2)): holds within a page, +C2 at each
    # boundary. Its steady-state stage is patched below from BYPASS(CURR) to
    # SUBTRACT(CURR, One) so it counts -1 per element; with C2 = 195+PSTEP the
    # boundary step lands the value at (page s, elem j) on -j + s*PSTEP.
    ideg = Scan(AluOp.ADD, Zero, init=One, _subdim_step=C2)
    t2 = q + ideg
    # outer max-scan over an expr that contains the ideg scan:
    # Scan.__post_init__ rejects the composition, so construct unchecked.
    m = object.__new__(Scan)
    object.__setattr__(m, "op", AluOp.MAX)
    object.__setattr__(m, "expr", t2)
    object.__setattr__(m, "init", None)
    object.__setattr__(m, "_subdim_step", None)
    spec = Spec(body=m, reference=_ref)

    def patch(uops):
        assert len(uops) == 3, f"expected [seed, steady, step], got {len(uops)}"
        seed, steady, _step = uops
        ks = [i for i, d in enumerate(steady.datapath_config)
              if d.op == AluOp.BYPASS and d.alu_src0 == AluInp.CURR_ALU_OUT]
        assert len(ks) == 1, f"page-counter stage not unique: {ks}"
        k = ks[0]
        one_lane = seed.datapath_config[k].alu_src0  # delay lane carrying One
        assert one_lane.name.startswith("PREV_DELAY"), one_lane
        steady.datapath_config[k].op = AluOp.SUBTRACT
        steady.datapath_config[k].alu_src0 = AluInp.CURR_ALU_OUT
        steady.datapath_config[k].alu_src1 = one_lane

    return _register_op("QPACK_ROWMAX_ANT", spec, subdim=True, patch=patch)


def _register_tail_ops():
    """Three ops collapsing the per-column decode chain:
      MODJ_ANT:  E -> j      j = 256*(wc>0) - wc, wc = E - 256*round(E/256)
      DECX_ANT:  j -> x      cand = round(j/14); xr = j - 14*cand;
                             x = xr + 14*(xr<0)
      DECY_ANT:  j -> y      y = cand - (xr<0)
    """
    from concourse.dve_spec import Spec, Src0, C0, C1, C2, Zero
    from concourse.dve_uop import AluOp

    def _wc(E):
        rn = np.float32(np.float32(E * np.float32(2.0 ** -8)) + np.float32(M3)
                        ) - np.float32(M3)
        return np.float32(E - np.float32(256.0) * rn)

    def _modj_ref(in0, in1, c0, c1, c2):
        wc = _wc(in0.astype(np.float32))
        return np.float32(256.0 * (wc > 0) - wc)

    def _cand_xr(j):
        cand = np.float32(np.float32(j * np.float32(1.0 / 14.0))
                          + np.float32(M3)) - np.float32(M3)
        xr = np.float32(j - 14.0 * cand)
        return cand, xr

    def _decx_ref(in0, in1, c0, c1, c2):
        cand, xr = _cand_xr(in0.astype(np.float32))
        return np.float32(xr + 14.0 * (xr < 0))

    def _decy_ref(in0, in1, c0, c1, c2):
        cand, xr = _cand_xr(in0.astype(np.float32))
        return np.float32(cand - 1.0 * (xr < 0))

    # MODJ: c0=2^-8, c1=M3, c2=-256; 256 is the hoisted const Zero-C2
    t1 = (Src0 * C0) + C1
    rn = t1 - C1
    wc = Src0 + (rn * C2)
    j = ((wc > Zero) * (Zero - C2)) - wc
    modj = _register_op("MODJ_ANT", Spec(body=j, reference=_modj_ref),
                        subdim=False)

    # DECX: c0=1/14, c1=M3, c2=-14
    c14 = (Src0 * C0) + C1
    cand = c14 - C1
    xr = Src0 + (cand * C2)
    x = xr - ((xr < Zero) * C2)
    decx = _register_op("DECX_ANT", Spec(body=x, reference=_decx_ref),
                        subdim=False)

    # DECY: same chain, output cand - (xr<0)
    y = cand - (xr < Zero)
    decy = _register_op("DECY_ANT", Spec(body=y, reference=_decy_ref),
                        subdim=False)
    return modj, decx, decy


def _build():
    import concourse.bacc as bacc
    import concourse.mybir as mybir
    from concourse.tile import TileContext

    qpack = _register_qpack()
    modj, decx, decy = _register_tail_ops()

    F32 = mybir.dt.float32
    A = mybir.AluOpType
    AF = mybir.ActivationFunctionType

    rows = N_TILES * ROWS_PER_TILE
    ncols = N_TILES * K_PER_PART   # 224

    nc = bacc.Bacc()
    h = nc.declare_dram_parameter("h", [rows, NPIX], F32, isOutput=False)
    tx = nc.declare_dram_parameter("tx", [128, ncols], F32, isOutput=False)
    ty = nc.declare_dram_parameter("ty", [128, ncols], F32, isOutput=False)
    out = nc.declare_dram_parameter("part", [128, 2 * 4], F32, isOutput=True)

    n_chunks = 4
    cw = ncols // n_chunks
    tiles_per_chunk = N_TILES // n_chunks

    with TileContext(nc) as tc:
        with tc.tile_pool(name="hpool", bufs=12) as hpool, \
             tc.tile_pool(name="consts", bufs=1) as cpool, \
             tc.tile_pool(name="acc", bufs=1) as accpool:
            kmax = accpool.tile([128, ncols], F32, tag="kmax")
            part_sb = accpool.tile([128, 2 * n_chunks], F32, tag="part")

            # targets ride the scalar-engine HWDGE ring so they never queue
            # behind the h stream on the sync ring
            txt = cpool.tile([128, ncols], F32, tag="txt")
            tyt = cpool.tile([128, ncols], F32, tag="tyt")

            def emit_tail_chunk(c):
                lo, hi = c * cw, (c + 1) * cw
                km = kmax[:, lo:hi]

                def tile(tag):
                    return accpool.tile([128, cw], F32, name=f"{tag}_{c}",
                                        tag=f"{tag}_{c}")

                jt = tile("jt")
                nc.vector._custom_dve(modj, out=jt[:], in0=km,
                                      s0=2.0 ** -8, s1=M3, imm2=-256.0)
                xt = tile("xt")
                nc.vector._custom_dve(decx, out=xt[:], in0=jt[:],
                                      s0=1.0 / 14.0, s1=M3, imm2=-14.0)
                yt = tile("yt")
                nc.vector._custom_dve(decy, out=yt[:], in0=jt[:],
                                      s0=1.0 / 14.0, s1=M3, imm2=-14.0)
                dx = tile("dx")
                nc.vector.scalar_tensor_tensor(dx[:], xt[:], 0.0625,
                                               txt[:, lo:hi],
                                               op0=A.mult, op1=A.subtract)
                dy = tile("dy")
                nc.vector.scalar_tensor_tensor(dy[:], yt[:], 0.0625,
                                               tyt[:, lo:hi],
                                               op0=A.mult, op1=A.subtract)
                sq = accpool.tile([128, cw], F32, tag=f"sq_{c}")
                nc.scalar.activation(sq[:], dx[:], AF.Square,
                                     accum_out=part_sb[:, 2 * c:2 * c + 1])
                sq2 = accpool.tile([128, cw], F32, tag=f"sq2_{c}")
                nc.scalar.activation(sq2[:], dy[:], AF.Square,
                                     accum_out=part_sb[:, 2 * c + 1:2 * c + 2])

            nc.scalar.dma_start(txt[:], tx[:])
            nc.scalar.dma_start(tyt[:], ty[:])

            # the last tile streams as two k-halves (k is the fast DRAM index,
            # so each half is still 5488 contiguous bytes per partition),
            # halving the serial QPACK after the final DMA byte lands
            work = [(t, 0, K_PER_PART) for t in range(N_TILES - 1)]
            work += [(N_TILES - 1, 0, 5), (N_TILES - 1, 5, 5),
                     (N_TILES - 1, 10, 4)]
            for t, k0, nk in work:
                ht = hpool.tile([128, nk * NPIX], F32, tag="ht")
                nc.sync.dma_start(
                    ht[:],
                    h[t * ROWS_PER_TILE:(t + 1) * ROWS_PER_TILE, :]
                    .rearrange("(p k) f -> p (k f)", p=128)
                    [:, k0 * NPIX:(k0 + nk) * NPIX])
                ht3 = ht[:].rearrange("p (s n) -> p s n", n=NPIX)
                nc.vector._custom_dve(
                    qpack, out=ht3, in0=ht3,
                    s0=SCALE, s1=MAGIC, imm2=float(NPIX) - 1.0 + PSTEP)
                # running max at each row's last element = that row's packed max
                nc.scalar.activation(
                    kmax[:, t * K_PER_PART + k0:t * K_PER_PART + k0 + nk],
                    ht3[:, :, NPIX - 1:NPIX].rearrange("p s o -> p (s o)"),
                    AF.Identity)
                if k0 == 0 and (t + 1) % tiles_per_chunk == 0 and t + 1 < N_TILES:
                    emit_tail_chunk((t + 1) // tiles_per_chunk - 1)
            emit_tail_chunk(n_chunks - 1)

            nc.sync.dma_start(out[:], part_sb[:])
    nc.finalize()
    return nc


def _targets(t_shard: np.ndarray):
    """Per-column raw targets matching kmax layout (col = tile*14 + joint)."""
    b = t_shard.shape[0]
    t2 = t_shard.reshape(b, 28)
    tpx = np.ascontiguousarray(t2[:, :14]).reshape(-1)
    tpy = np.ascontiguousarray(t2[:, 14:]).reshape(-1)
    tx = tpx.reshape(N_TILES, 128, K_PER_PART).transpose(1, 0, 2).reshape(128, -1)
    ty = tpy.reshape(N_TILES, 128, K_PER_PART).transpose(1, 0, 2).reshape(128, -1)
    return np.ascontiguousarray(tx), np.ascontiguousarray(ty)


def kernel(o: np.ndarray, h: np.ndarray, t: np.ndarray, v: np.ndarray,
           _trace: bool = False, _tmpdir: str | None = None) -> np.ndarray:
    from concourse.bass_utils import run_bass_kernel_spmd

    if "nc" not in _STATE:
        _STATE["nc"] = _build()
    nc = _STATE["nc"]

    h = np.ascontiguousarray(np.asarray(h, dtype=np.float32))
    t = np.ascontiguousarray(np.asarray(t, dtype=np.float32))
    bs = B // N_CORES
    in_maps = []
    for c in range(N_CORES):
        h_shard = h[c * bs:(c + 1) * bs].reshape(bs * NJ, NPIX)
        txv, tyv = _targets(t[c * bs:(c + 1) * bs])
        in_maps.append({"h": h_shard, "tx": txv, "ty": tyv})

    res = run_bass_kernel_spmd(
        nc, in_maps, list(range(N_CORES)),
        trace=_trace, tmpdir=_tmpdir)
    _STATE["last_result"] = res
    total = np.float64(0.0)
    for c in range(N_CORES):
        total += np.asarray(res.results[c]["part"], dtype=np.float64).sum()
    n = np.float32(B * NJ)
    return np.float32(np.float32(total) / n)


# revision 40
# speedup vs baseline: 1.0519x; 1.0519x over previous
"""Trainium2 kernel for nn_MeanSquaredError2: MSE between argmax-decoded
heatmap coordinates and targets.

loss = sum_{b,j} [(px - tpx)^2 + (py - tpy)^2] / (B*NJ)
  where idx = argmax(h[b,j]), px = (idx%14)/16, py = (idx//14)/16 and
  (tpx, tpy) follow the reference's concat-then-reshape pairing of t.
Inputs o and v do not affect the result (USE_VISIBILITY=False).

Pure data parallel over 8 cores (2048 batches each). Per core, h streams in
16 tiles of [128 part x (14 rows x 196 pix)]; the whole
quantize+pack+row-argmax runs as ONE single-stream custom DVE instruction
per tile (~1.07 cyc/elem), leaving the kernel DMA-bound:

  custom op QPACK_ROWMAX_ANT (registered into dve_ops.OPS at build time):
      u  = (h*2^16 + MAGIC)        # rounds h*2^16 to the 256-grid
      q  = u - MAGIC               # exact (Sterbenz): q = 256*n
      k  = (q - Idx) + PageIdx(0, 196+2^20)
                                   # == q - j + s*2^20 for in-page pixel j,
                                   #    row (joint) s; all exact (< 2^24)
      out = running_max(k)         # inclusive scan, no page reset needed:
                                   # s*2^20 > range(q-j) isolates rows

  The position rides in k's low byte (-j mod 256) and the row offset above
  the value bits, so the running max at element s*196+195 equals row s's
  packed max; that element is extracted per row (ACT copy, strided). The
  "-j + s*PSTEP" counter is one page-counter scan whose steady state is
  hand-patched from BYPASS(CURR) to SUBTRACT(CURR, One) (the documented
  UopConfig escape hatch), and the outer max-scan's expr contains that scan
  - Scan.__post_init__ forbids the composition, but lower() schedules it
  correctly; both validated against numpy on hardware.

Tail (4 chunks of [128,56], interleaved with the stream; 3 more custom DVE
ops): MODJ_ANT: wc = E - 256*round(E/256), j = 256*(wc>0) - wc;
DECX_ANT: cand = round(j/14), xr = j - 14*cand, x = xr + 14*(xr<0);
DECY_ANT: y = cand - (xr<0); then dx = x/16 - tx, dy = y/16 - ty (DVE stt)
and ACT Square+accum per partition. The last h tile streams as two k-halves
to halve the final serial QPACK. Host sums 8x[128,8] partials / N.

Quantization is 2^-8 (vs jnp.argmax exact): ~1% of rows hit top-2 ties and
may decode the runner-up pixel; the loss deltas are zero-mean and mostly
cancel, measured rel err ~1e-3 (threshold 2e-2).
"""
import numpy as np

B = 16384
NJ = 14
NPIX = 196
N_CORES = 8
ROWS_PER_TILE = 1792          # 128 partitions x 14 rows
K_PER_PART = 14
N_TILES = 16                  # (B/N_CORES)*NJ / ROWS_PER_TILE

SCALE = float(2 ** 16)
MAGIC = 1.5 * 2 ** 32         # ulp = 256 -> q on the 256-grid
M3 = 1.5 * 2 ** 23            # ulp = 1   -> round-to-integer magic
PSTEP = float(2 ** 20)        # per-row offset; > range(q - j) ~ 2*6.3*2^16

_STATE = {}


def _register_op(name, spec, subdim, patch=None):
    """Register one custom DVE op into dve_ops.OPS at runtime; `patch` may
    hand-edit the lowered uop list (the documented escape hatch for programs
    `lower()` cannot express directly)."""
    import concourse.dve_ops as dve_ops
    from concourse.dve_ops import DveOp
    from concourse.dve_spec import _has_src1 as has_src1
    from concourse.dve_uop import DveOpSpec

    if name in dve_ops._SUB_OPCODE_FOR_NAME:
        return next(op for op in dve_ops.OPS if op.name == name)
    row = dve_ops._CUSTOM_DVE_ROW_BASE + len(dve_ops.OPS)
    assert row < 0x20, "custom-DVE opcode rows exhausted"
    from concourse.dve_spec import lower
    shas = {}
    compiled = {}
    for ver in ("v3", "v4"):
        uops = lower(spec, ver=ver)
        if patch is not None:
            patch(uops)
        s = DveOpSpec(name=name, opcode=row, uops=uops, rd1_en=has_src1(spec))
        shas[ver] = s.sha(ver)
        compiled[ver] = s
    op = DveOp(name, spec, subdim=subdim, uops_sha=shas)
    dve_ops.OPS.append(op)
    dve_ops._SUB_OPCODE_FOR_NAME[name] = row
    dve_ops.CUSTOM_DVE_SPECS[name] = spec
    for ver, s in compiled.items():
        dve_ops._COMPILE_CACHE[(name, ver)] = s
    return op


def _register_qpack():
    from concourse.dve_spec import (
        Spec, Src0, C0, C1, C2, Zero, One, Scan,
    )
    from concourse.dve_uop import AluOp, AluInp

    def _ref(in0, in1, c0, c1, c2):
        p = in0.shape[0]
        flat = in0.reshape(p, -1)
        n = flat.shape[1]
        u = (flat.astype(np.float32) * np.float32(c0)).astype(np.float32)
        u = (u + np.float32(c1)).astype(np.float32)
        q = (u - np.float32(c1)).astype(np.float32)
        j = (np.arange(n) % NPIX).astype(np.float32)
        s = (np.arange(n) // NPIX).astype(np.float32) * np.float32(c2 - NPIX + 1)
        k = (q + (s - j).astype(np.float32)).astype(np.float32)
        return np.maximum.accumulate(k, axis=1).reshape(in0.shape)

    u = (Src0 * C0) + C1
    q = u - C1
    # page-counter scan (PageIdx(One, C2)): holds within a page, +C2 at each
    # boundary. Its steady-state stage is patched below from BYPASS(CURR) to
    # SUBTRACT(CURR, One) so it counts -1 per element; with C2 = 195+PSTEP the
    # boundary step lands the value at (page s, elem j) on -j + s*PSTEP.
    ideg = Scan(AluOp.ADD, Zero, init=One, _subdim_step=C2)
    t2 = q + ideg
    # outer max-scan over an expr that contains the ideg scan:
    # Scan.__post_init__ rejects the composition, so construct unchecked.
    m = object.__new__(Scan)
    object.__setattr__(m, "op", AluOp.MAX)
    object.__setattr__(m, "expr", t2)
    object.__setattr__(m, "init", None)
    object.__setattr__(m, "_subdim_step", None)
    spec = Spec(body=m, reference=_ref)

    def patch(uops):
        assert len(uops) == 3, f"expected [seed, steady, step], got {len(uops)}"
        seed, steady, _step = uops
        ks = [i for i, d in enumerate(steady.datapath_config)
              if d.op == AluOp.BYPASS and d.alu_src0 == AluInp.CURR_ALU_OUT]
        assert len(ks) == 1, f"page-counter stage not unique: {ks}"
        k = ks[0]
        one_lane = seed.datapath_config[k].alu_src0  # delay lane carrying One
        assert one_lane.name.startswith("PREV_DELAY"), one_lane
        steady.datapath_config[k].op = AluOp.SUBTRACT
        steady.datapath_config[k].alu_src0 = AluInp.CURR_ALU_OUT
        steady.datapath_config[k].alu_src1 = one_lane

    return _register_op("QPACK_ROWMAX_ANT", spec, subdim=True, patch=patch)


def _register_tail_ops():
    """Three ops collapsing the per-column decode chain:
      MODJ_ANT:  E -> j      j = 256*(wc>0) - wc, wc = E - 256*round(E/256)
      DECX_ANT:  j -> x      cand = round(j/14); xr = j - 14*cand;
                             x = xr + 14*(xr<0)
      DECY_ANT:  j -> y      y = cand - (xr<0)
    """
    from concourse.dve_spec import Spec, Src0, C0, C1, C2, Zero
    from concourse.dve_uop import AluOp

    def _wc(E):
        rn = np.float32(np.float32(E * np.float32(2.0 ** -8)) + np.float32(M3)
                        ) - np.float32(M3)
        return np.float32(E - np.float32(256.0) * rn)

    def _modj_ref(in0, in1, c0, c1, c2):
        wc = _wc(in0.astype(np.float32))
        return np.float32(256.0 * (wc > 0) - wc)

    def _cand_xr(j):
        cand = np.float32(np.float32(j * np.float32(1.0 / 14.0))
                          + np.float32(M3)) - np.float32(M3)
        xr = np.float32(j - 14.0 * cand)
        return cand, xr

    def _decx_ref(in0, in1, c0, c1, c2):
        cand, xr = _cand_xr(in0.astype(np.float32))
        return np.float32(xr + 14.0 * (xr < 0))

    def _decy_ref(in0, in1, c0, c1, c2):
        cand, xr = _cand_xr(in0.astype(np.float32))
        return np.float32(cand - 1.0 * (xr < 0))

    # MODJ: c0=2^-8, c1=M3, c2=-256; 256 is the hoisted const Zero-C2
    t1 = (Src0 * C0) + C1
    rn = t1 - C1
    wc = Src0 + (rn * C2)
    j = ((wc > Zero) * (Zero - C2)) - wc
    modj = _register_op("MODJ_ANT", Spec(body=j, reference=_modj_ref),
                        subdim=False)

    # DECX: c0=1/14, c1=M3, c2=-14
    c14 = (Src0 * C0) + C1
    cand = c14 - C1
    xr = Src0 + (cand * C2)
    x = xr - ((xr < Zero) * C2)
    decx = _register_op("DECX_ANT", Spec(body=x, reference=_decx_ref),
                        subdim=False)

    # DECY: same chain, output cand - (xr<0)
    y = cand - (xr < Zero)
    decy = _register_op("DECY_ANT", Spec(body=y, reference=_decy_ref),
                        subdim=False)
    return modj, decx, decy


def _build():
    import concourse.bacc as bacc
    import concourse.mybir as mybir
    from concourse.tile import TileContext

    qpack = _register_qpack()
    modj, decx, decy = _register_tail_ops()

    F32 = mybir.dt.float32
    A = mybir.AluOpType
    AF = mybir.ActivationFunctionType

    rows = N_TILES * ROWS_PER_TILE
    ncols = N_TILES * K_PER_PART   # 224

    nc = bacc.Bacc()
    h = nc.declare_dram_parameter("h", [rows, NPIX], F32, isOutput=False)
    tx = nc.declare_dram_parameter("tx", [128, ncols], F32, isOutput=False)
    ty = nc.declare_dram_parameter("ty", [128, ncols], F32, isOutput=False)
    out = nc.declare_dram_parameter("part", [128, 2 * 4], F32, isOutput=True)

    n_chunks = 4
    cw = ncols // n_chunks
    tiles_per_chunk = N_TILES // n_chunks

    with TileContext(nc) as tc:
        with tc.tile_pool(name="hpool", bufs=12) as hpool, \
             tc.tile_pool(name="consts", bufs=1) as cpool, \
             tc.tile_pool(name="acc", bufs=1) as accpool:
            kmax = accpool.tile([128, ncols], F32, tag="kmax")
            part_sb = accpool.tile([128, 2 * n_chunks], F32, tag="part")

            # targets ride the scalar-engine HWDGE ring so they never queue
            # behind the h stream on the sync ring
            txt = cpool.tile([128, ncols], F32, tag="txt")
            tyt = cpool.tile([128, ncols], F32, tag="tyt")

            def emit_tail_chunk(c):
                lo, hi = c * cw, (c + 1) * cw
                km = kmax[:, lo:hi]

                def tile(tag):
                    return accpool.tile([128, cw], F32, name=f"{tag}_{c}",
                                        tag=f"{tag}_{c}")

                jt = tile("jt")
                nc.vector._custom_dve(modj, out=jt[:], in0=km,
                                      s0=2.0 ** -8, s1=M3, imm2=-256.0)
                xt = tile("xt")
                nc.vector._custom_dve(decx, out=xt[:], in0=jt[:],
                                      s0=1.0 / 14.0, s1=M3, imm2=-14.0)
                yt = tile("yt")
                nc.vector._custom_dve(decy, out=yt[:], in0=jt[:],
                                      s0=1.0 / 14.0, s1=M3, imm2=-14.0)
                dx = tile("dx")
                nc.vector.scalar_tensor_tensor(dx[:], xt[:], 0.0625,
                                               txt[:, lo:hi],
                                               op0=A.mult, op1=A.subtract)
                dy = tile("dy")
                nc.vector.scalar_tensor_tensor(dy[:], yt[:], 0.0625,
                                               tyt[:, lo:hi],
                                               op0=A.mult, op1=A.subtract)
                sq = accpool.tile([128, cw], F32, tag=f"sq_{c}")
                nc.scalar.activation(sq[:], dx[:], AF.Square,
                                     accum_out=part_sb[:, 2 * c:2 * c + 1])
                sq2 = accpool.tile([128, cw], F32, tag=f"sq2_{c}")
                nc.scalar.activation(sq2[:], dy[:], AF.Square,
                                     accum_out=part_sb[:, 2 * c + 1:2 * c + 2])

            nc.scalar.dma_start(txt[:], tx[:])
            nc.scalar.dma_start(tyt[:], ty[:])

            # the last tile streams as two k-halves (k is the fast DRAM index,
            # so each half is still 5488 contiguous bytes per partition),
            # halving the serial QPACK after the final DMA byte lands
            work = [(t, 0, K_PER_PART) for t in range(N_TILES - 1)]
            work += [(N_TILES - 1, 0, 7), (N_TILES - 1, 7, 7)]
            for t, k0, nk in work:
                ht = hpool.tile([128, nk * NPIX], F32, tag="ht")
                nc.sync.dma_start(
                    ht[:],
                    h[t * ROWS_PER_TILE:(t + 1) * ROWS_PER_TILE, :]
                    .rearrange("(p k) f -> p (k f)", p=128)
                    [:, k0 * NPIX:(k0 + nk) * NPIX])
                ht3 = ht[:].rearrange("p (s n) -> p s n", n=NPIX)
                nc.vector._custom_dve(
                    qpack, out=ht3, in0=ht3,
                    s0=SCALE, s1=MAGIC, imm2=float(NPIX) - 1.0 + PSTEP)
                # running max at each row's last element = that row's packed max
                nc.scalar.activation(
                    kmax[:, t * K_PER_PART + k0:t * K_PER_PART + k0 + nk],
                    ht3[:, :, NPIX - 1:NPIX].rearrange("p s o -> p (s o)"),
                    AF.Identity)
                if k0 == 0 and (t + 1) % tiles_per_chunk == 0 and t + 1 < N_TILES:
                    emit_tail_chunk((t + 1) // tiles_per_chunk - 1)
            emit_tail_chunk(n_chunks - 1)

            nc.sync.dma_start(out[:], part_sb[:])
    nc.finalize()
    return nc


def _targets(t_shard: np.ndarray):
    """Per-column raw targets matching kmax layout (col = tile*14 + joint)."""
    b = t_shard.shape[0]
    t2 = t_shard.reshape(b, 28)
    tpx = np.ascontiguousarray(t2[:, :14]).reshape(-1)
    tpy = np.ascontiguousarray(t2[:, 14:]).reshape(-1)
    tx = tpx.reshape(N_TILES, 128, K_PER_PART).transpose(1, 0, 2).reshape(128, -1)
    ty = tpy.reshape(N_TILES, 128, K_PER_PART).transpose(1, 0, 2).reshape(128, -1)
    return np.ascontiguousarray(tx), np.ascontiguousarray(ty)


def kernel(o: np.ndarray, h: np.ndarray, t: np.ndarray, v: np.ndarray,
           _trace: bool = False, _tmpdir: str | None = None) -> np.ndarray:
    from concourse.bass_utils import run_bass_kernel_spmd

    if "nc" not in _STATE:
        _STATE["nc"] = _build()
    nc = _STATE["nc"]

    h = np.ascontiguousarray(np.asarray(h, dtype=np.float32))
    t = np.ascontiguousarray(np.asarray(t, dtype=np.float32))
    bs = B // N_CORES
    in_maps = []
    for c in range(N_CORES):
        h_shard = h[c * bs:(c + 1) * bs].reshape(bs * NJ, NPIX)
        txv, tyv = _targets(t[c * bs:(c + 1) * bs])
        in_maps.append({"h": h_shard, "tx": txv, "ty": tyv})

    res = run_bass_kernel_spmd(
        nc, in_maps, list(range(N_CORES)),
        trace=_trace, tmpdir=_tmpdir)
    _STATE["last_result"] = res
    total = np.float64(0.0)
    for c in range(N_CORES):
        total += np.asarray(res.results[c]["part"], dtype=np.float64).sum()
    n = np.float32(B * NJ)
    return np.float32(np.float32(total) / n)


# revision 44
# speedup vs baseline: 1.2107x; 1.1510x over previous
"""Trainium2 kernel for nn_MeanSquaredError2: MSE between argmax-decoded
heatmap coordinates and targets.

loss = sum_{b,j} [(px - tpx)^2 + (py - tpy)^2] / (B*NJ)
  where idx = argmax(h[b,j]), px = (idx%14)/16, py = (idx//14)/16 and
  (tpx, tpy) follow the reference's concat-then-reshape pairing of t.
Inputs o and v do not affect the result (USE_VISIBILITY=False).

Pure data parallel over 8 cores (2048 batches each). Per core, h streams in
16 tiles of [128 part x (14 rows x 196 pix)]; the whole
quantize+pack+row-argmax runs as ONE single-stream custom DVE instruction
per tile (~1.07 cyc/elem), leaving the kernel DMA-bound:

  custom op QPACK_ROWMAX_ANT (registered into dve_ops.OPS at build time):
      u  = (h*2^16 + MAGIC)        # rounds h*2^16 to the 256-grid
      q  = u - MAGIC               # exact (Sterbenz): q = 256*n
      k  = (q - Idx) + PageIdx(0, 196+2^20)
                                   # == q - j + s*2^20 for in-page pixel j,
                                   #    row (joint) s; all exact (< 2^24)
      out = running_max(k)         # inclusive scan, no page reset needed:
                                   # s*2^20 > range(q-j) isolates rows

  The position rides in k's low byte (-j mod 256) and the row offset above
  the value bits, so the running max at element s*196+195 equals row s's
  packed max; that element is extracted per row (ACT copy, strided). The
  "-j + s*PSTEP" counter is one page-counter scan whose steady state is
  hand-patched from BYPASS(CURR) to SUBTRACT(CURR, One) (the documented
  UopConfig escape hatch), and the outer max-scan's expr contains that scan
  - Scan.__post_init__ forbids the composition, but lower() schedules it
  correctly; both validated against numpy on hardware.

Tail (4 chunks of [128,56], interleaved with the stream; 3 more custom DVE
ops): MODJ_ANT: wc = E - 256*round(E/256), j = 256*(wc>0) - wc;
DECX_ANT: cand = round(j/14), xr = j - 14*cand, x = xr + 14*(xr<0);
DECY_ANT: y = cand - (xr<0); then dx = x/16 - tx, dy = y/16 - ty (DVE stt)
and ACT Square+accum per partition. The last h tile streams as two k-halves
to halve the final serial QPACK. Host sums 8x[128,8] partials / N.

Quantization is 2^-8 (vs jnp.argmax exact): ~1% of rows hit top-2 ties and
may decode the runner-up pixel; the loss deltas are zero-mean and mostly
cancel, measured rel err ~1e-3 (threshold 2e-2).
"""
import numpy as np

B = 16384
NJ = 14
NPIX = 196
N_CORES = 8
ROWS_PER_TILE = 1792          # 128 partitions x 14 rows
K_PER_PART = 14
N_TILES = 16                  # (B/N_CORES)*NJ / ROWS_PER_TILE

SCALE = float(2 ** 16)
MAGIC = 1.5 * 2 ** 32         # ulp = 256 -> q on the 256-grid
M3 = 1.5 * 2 ** 23            # ulp = 1   -> round-to-integer magic
PSTEP = float(2 ** 20)        # per-row offset; > range(q - j) ~ 2*6.3*2^16

_STATE = {}


def _register_op(name, spec, subdim, patch=None):
    """Register one custom DVE op into dve_ops.OPS at runtime; `patch` may
    hand-edit the lowered uop list (the documented escape hatch for programs
    `lower()` cannot express directly)."""
    import concourse.dve_ops as dve_ops
    from concourse.dve_ops import DveOp
    from concourse.dve_spec import _has_src1 as has_src1
    from concourse.dve_uop import DveOpSpec

    if name in dve_ops._SUB_OPCODE_FOR_NAME:
        return next(op for op in dve_ops.OPS if op.name == name)
    row = dve_ops._CUSTOM_DVE_ROW_BASE + len(dve_ops.OPS)
    assert row < 0x20, "custom-DVE opcode rows exhausted"
    from concourse.dve_spec import lower
    shas = {}
    compiled = {}
    for ver in ("v3", "v4"):
        uops = lower(spec, ver=ver)
        if patch is not None:
            patch(uops)
        s = DveOpSpec(name=name, opcode=row, uops=uops, rd1_en=has_src1(spec))
        shas[ver] = s.sha(ver)
        compiled[ver] = s
    op = DveOp(name, spec, subdim=subdim, uops_sha=shas)
    dve_ops.OPS.append(op)
    dve_ops._SUB_OPCODE_FOR_NAME[name] = row
    dve_ops.CUSTOM_DVE_SPECS[name] = spec
    for ver, s in compiled.items():
        dve_ops._COMPILE_CACHE[(name, ver)] = s
    return op


def _register_qpack():
    from concourse.dve_spec import (
        Spec, Src0, C0, C1, C2, Zero, One, Scan,
    )
    from concourse.dve_uop import AluOp, AluInp

    def _ref(in0, in1, c0, c1, c2):
        p = in0.shape[0]
        flat = in0.reshape(p, -1)
        n = flat.shape[1]
        u = (flat.astype(np.float32) * np.float32(c0)).astype(np.float32)
        u = (u + np.float32(c1)).astype(np.float32)
        q = (u - np.float32(c1)).astype(np.float32)
        j = (np.arange(n) % NPIX).astype(np.float32)
        s = (np.arange(n) // NPIX).astype(np.float32) * np.float32(c2 - NPIX + 1)
        k = (q + (s - j).astype(np.float32)).astype(np.float32)
        return np.maximum.accumulate(k, axis=1).reshape(in0.shape)

    u = (Src0 * C0) + C1
    q = u - C1
    # page-counter scan (PageIdx(One, C2)): holds within a page, +C2 at each
    # boundary. Its steady-state stage is patched below from BYPASS(CURR) to
    # SUBTRACT(CURR, One) so it counts -1 per element; with C2 = 195+PSTEP the
    # boundary step lands the value at (page s, elem j) on -j + s*PSTEP.
    ideg = Scan(AluOp.ADD, Zero, init=One, _subdim_step=C2)
    t2 = q + ideg
    # outer max-scan over an expr that contains the ideg scan:
    # Scan.__post_init__ rejects the composition, so construct unchecked.
    m = object.__new__(Scan)
    object.__setattr__(m, "op", AluOp.MAX)
    object.__setattr__(m, "expr", t2)
    object.__setattr__(m, "init", None)
    object.__setattr__(m, "_subdim_step", None)
    spec = Spec(body=m, reference=_ref)

    def patch(uops):
        assert len(uops) == 3, f"expected [seed, steady, step], got {len(uops)}"
        seed, steady, _step = uops
        ks = [i for i, d in enumerate(steady.datapath_config)
              if d.op == AluOp.BYPASS and d.alu_src0 == AluInp.CURR_ALU_OUT]
        assert len(ks) == 1, f"page-counter stage not unique: {ks}"
        k = ks[0]
        one_lane = seed.datapath_config[k].alu_src0  # delay lane carrying One
        assert one_lane.name.startswith("PREV_DELAY"), one_lane
        steady.datapath_config[k].op = AluOp.SUBTRACT
        steady.datapath_config[k].alu_src0 = AluInp.CURR_ALU_OUT
        steady.datapath_config[k].alu_src1 = one_lane

    return _register_op("QPACK_ROWMAX_ANT", spec, subdim=True, patch=patch)


def _register_tail_ops():
    """Three ops collapsing the per-column decode chain:
      MODJ_ANT:  E -> j      j = 256*(wc>0) - wc, wc = E - 256*round(E/256)
      DECX_ANT:  j -> x      cand = round(j/14); xr = j - 14*cand;
                             x = xr + 14*(xr<0)
      DECY_ANT:  j -> y      y = cand - (xr<0)
    """
    from concourse.dve_spec import Spec, Src0, C0, C1, C2, Zero
    from concourse.dve_uop import AluOp

    def _wc(E):
        rn = np.float32(np.float32(E * np.float32(2.0 ** -8)) + np.float32(M3)
                        ) - np.float32(M3)
        return np.float32(E - np.float32(256.0) * rn)

    def _modj_ref(in0, in1, c0, c1, c2):
        wc = _wc(in0.astype(np.float32))
        return np.float32(256.0 * (wc > 0) - wc)

    def _cand_xr(j):
        cand = np.float32(np.float32(j * np.float32(1.0 / 14.0))
                          + np.float32(M3)) - np.float32(M3)
        xr = np.float32(j - 14.0 * cand)
        return cand, xr

    def _decx_ref(in0, in1, c0, c1, c2):
        cand, xr = _cand_xr(in0.astype(np.float32))
        return np.float32(xr + 14.0 * (xr < 0))

    def _decy_ref(in0, in1, c0, c1, c2):
        cand, xr = _cand_xr(in0.astype(np.float32))
        return np.float32(cand - 1.0 * (xr < 0))

    # MODJ: c0=2^-8, c1=M3, c2=-256; 256 is the hoisted const Zero-C2
    t1 = (Src0 * C0) + C1
    rn = t1 - C1
    wc = Src0 + (rn * C2)
    j = ((wc > Zero) * (Zero - C2)) - wc
    modj = _register_op("MODJ_ANT", Spec(body=j, reference=_modj_ref),
                        subdim=False)

    # DECX: c0=1/14, c1=M3, c2=-14
    c14 = (Src0 * C0) + C1
    cand = c14 - C1
    xr = Src0 + (cand * C2)
    x = xr - ((xr < Zero) * C2)
    decx = _register_op("DECX_ANT", Spec(body=x, reference=_decx_ref),
                        subdim=False)

    # DECY: same chain, output cand - (xr<0)
    y = cand - (xr < Zero)
    decy = _register_op("DECY_ANT", Spec(body=y, reference=_decy_ref),
                        subdim=False)
    return modj, decx, decy


def _build():
    import concourse.bacc as bacc
    import concourse.mybir as mybir
    from concourse.tile import TileContext

    qpack = _register_qpack()
    modj, decx, decy = _register_tail_ops()

    F32 = mybir.dt.float32
    A = mybir.AluOpType
    AF = mybir.ActivationFunctionType

    rows = N_TILES * ROWS_PER_TILE
    ncols = N_TILES * K_PER_PART   # 224

    nc = bacc.Bacc()
    h = nc.declare_dram_parameter("h", [rows, NPIX], F32, isOutput=False)
    tx = nc.declare_dram_parameter("tx", [128, ncols], F32, isOutput=False)
    ty = nc.declare_dram_parameter("ty", [128, ncols], F32, isOutput=False)
    out = nc.declare_dram_parameter("part", [128, 10], F32, isOutput=True)

    # tail chunk column ranges; the last is tiny (the final 4-row piece) so
    # the post-stream serial chain is as short as possible
    chunk_bounds = [(0, 56), (56, 112), (112, 168), (168, 220), (220, 224)]
    n_chunks = len(chunk_bounds)

    with TileContext(nc) as tc:
        with tc.tile_pool(name="hpool", bufs=12) as hpool, \
             tc.tile_pool(name="consts", bufs=1) as cpool, \
             tc.tile_pool(name="acc", bufs=1) as accpool:
            kmax = accpool.tile([128, ncols], F32, tag="kmax")
            part_sb = accpool.tile([128, 2 * n_chunks], F32, tag="part")

            # targets ride the scalar-engine HWDGE ring so they never queue
            # behind the h stream on the sync ring
            txt = cpool.tile([128, ncols], F32, tag="txt")
            tyt = cpool.tile([128, ncols], F32, tag="tyt")

            def emit_tail_chunk(c, dve_square=False):
                lo, hi = chunk_bounds[c]
                cw = hi - lo
                km = kmax[:, lo:hi]

                def tile(tag):
                    return accpool.tile([128, cw], F32, name=f"{tag}_{c}",
                                        tag=f"{tag}_{c}")

                jt = tile("jt")
                nc.vector._custom_dve(modj, out=jt[:], in0=km,
                                      s0=2.0 ** -8, s1=M3, imm2=-256.0)
                xt = tile("xt")
                nc.vector._custom_dve(decx, out=xt[:], in0=jt[:],
                                      s0=1.0 / 14.0, s1=M3, imm2=-14.0)
                yt = tile("yt")
                nc.vector._custom_dve(decy, out=yt[:], in0=jt[:],
                                      s0=1.0 / 14.0, s1=M3, imm2=-14.0)
                dx = tile("dx")
                nc.vector.scalar_tensor_tensor(dx[:], xt[:], 0.0625,
                                               txt[:, lo:hi],
                                               op0=A.mult, op1=A.subtract)
                dy = tile("dy")
                nc.vector.scalar_tensor_tensor(dy[:], yt[:], 0.0625,
                                               tyt[:, lo:hi],
                                               op0=A.mult, op1=A.subtract)
                if dve_square:
                    # final chunk: stay on DVE end-to-end (no cross-engine
                    # sem hops on the post-stream critical path)
                    sq = tile("sq")
                    nc.vector.tensor_tensor(sq[:], dx[:], dx[:], op=A.mult)
                    nc.vector.tensor_reduce(
                        part_sb[:, 2 * c:2 * c + 1], sq[:],
                        axis=mybir.AxisListType.X, op=A.add)
                    sq2 = tile("sq2")
                    nc.vector.tensor_tensor(sq2[:], dy[:], dy[:], op=A.mult)
                    nc.vector.tensor_reduce(
                        part_sb[:, 2 * c + 1:2 * c + 2], sq2[:],
                        axis=mybir.AxisListType.X, op=A.add)
                else:
                    sq = tile("sq")
                    nc.scalar.activation(sq[:], dx[:], AF.Square,
                                         accum_out=part_sb[:, 2 * c:2 * c + 1])
                    sq2 = tile("sq2")
                    nc.scalar.activation(
                        sq2[:], dy[:], AF.Square,
                        accum_out=part_sb[:, 2 * c + 1:2 * c + 2])

            nc.scalar.dma_start(txt[:], tx[:])
            nc.scalar.dma_start(tyt[:], ty[:])

            # the last tile streams as two k-pieces (k is the fast DRAM index,
            # so each piece stays contiguous per partition); the final 4-row
            # piece minimizes the serial QPACK after the last DMA byte lands.
            # chunk_after maps work-item index -> tail chunk to emit.
            work = [(t, 0, K_PER_PART) for t in range(N_TILES - 1)]
            work += [(N_TILES - 1, 0, 10), (N_TILES - 1, 10, 4)]
            chunk_after = {3: 0, 7: 1, 11: 2, 15: 3}
            for i, (t, k0, nk) in enumerate(work):
                ht = hpool.tile([128, nk * NPIX], F32, tag="ht")
                nc.sync.dma_start(
                    ht[:],
                    h[t * ROWS_PER_TILE:(t + 1) * ROWS_PER_TILE, :]
                    .rearrange("(p k) f -> p (k f)", p=128)
                    [:, k0 * NPIX:(k0 + nk) * NPIX])
                ht3 = ht[:].rearrange("p (s n) -> p s n", n=NPIX)
                nc.vector._custom_dve(
                    qpack, out=ht3, in0=ht3,
                    s0=SCALE, s1=MAGIC, imm2=float(NPIX) - 1.0 + PSTEP)
                # running max at each row's last element = that row's packed
                # max; the final piece extracts on DVE (post-stream critical
                # path stays on one engine), the rest on the idle ACT
                ext_src = ht3[:, :, NPIX - 1:NPIX].rearrange("p s o -> p (s o)")
                ext_dst = kmax[:, t * K_PER_PART + k0:t * K_PER_PART + k0 + nk]
                if i == len(work) - 1:
                    nc.vector.tensor_copy(ext_dst, ext_src)
                else:
                    nc.scalar.activation(ext_dst, ext_src, AF.Identity)
                if i in chunk_after:
                    emit_tail_chunk(chunk_after[i])
            emit_tail_chunk(n_chunks - 1, dve_square=True)

            nc.sync.dma_start(out[:], part_sb[:])
    nc.finalize()
    return nc


def _targets(t_shard: np.ndarray):
    """Per-column raw targets matching kmax layout (col = tile*14 + joint)."""
    b = t_shard.shape[0]
    t2 = t_shard.reshape(b, 28)
    tpx = np.ascontiguousarray(t2[:, :14]).reshape(-1)
    tpy = np.ascontiguousarray(t2[:, 14:]).reshape(-1)
    tx = tpx.reshape(N_TILES, 128, K_PER_PART).transpose(1, 0, 2).reshape(128, -1)
    ty = tpy.reshape(N_TILES, 128, K_PER_PART).transpose(1, 0, 2).reshape(128, -1)
    return np.ascontiguousarray(tx), np.ascontiguousarray(ty)


def kernel(o: np.ndarray, h: np.ndarray, t: np.ndarray, v: np.ndarray,
           _trace: bool = False, _tmpdir: str | None = None) -> np.ndarray:
    from concourse.bass_utils import run_bass_kernel_spmd

    if "nc" not in _STATE:
        _STATE["nc"] = _build()
    nc = _STATE["nc"]

    h = np.ascontiguousarray(np.asarray(h, dtype=np.float32))
    t = np.ascontiguousarray(np.asarray(t, dtype=np.float32))
    bs = B // N_CORES
    in_maps = []
    for c in range(N_CORES):
        h_shard = h[c * bs:(c + 1) * bs].reshape(bs * NJ, NPIX)
        txv, tyv = _targets(t[c * bs:(c + 1) * bs])
        in_maps.append({"h": h_shard, "tx": txv, "ty": tyv})

    res = run_bass_kernel_spmd(
        nc, in_maps, list(range(N_CORES)),
        trace=_trace, tmpdir=_tmpdir)
    _STATE["last_result"] = res
    total = np.float64(0.0)
    for c in range(N_CORES):
        total += np.asarray(res.results[c]["part"], dtype=np.float64).sum()
    n = np.float32(B * NJ)
    return np.float32(np.float32(total) / n)
